# revision 19
# baseline (speedup 1.0000x reference)
"""DEC soft-assignment (student-t, row-normalized) Trainium2 Bass kernel.

q[n,k] = (1 + ||x_n - c_k||^2/alpha)^(-(alpha+1)/2), row-normalized.

Strategy (8 cores, data-parallel over N), DMA-roofline oriented
(~160us vs ~121us pure-DMA floor for the 42MB/core of HBM traffic):
  u = 1 + (||x||^2 - 2 x.c + ||c||^2)/alpha assembled fully in PSUM:
  - emb tiles stream in via gpsimd SWDGE DMA casting fp32->bf16
    in-flight (free cast), 8-tile prefetch so the input stream is hot
    from t=0 and rides through the centers setup.
  - centersT built once at setup: [128, 6, 512] fp8e4 chunk-major,
    scaled by -2/alpha; csq gathered per-block into a [1,512] row via
    SBUF->SBUF DMAs (no DRAM roundtrip); cs2 = [ones; 1+csq/a] bf16
    (partition-1 row written via a tiny SBUF->SBUF DMA hop).
  - per 128-row tile, two software-pipelined stages (stage2 lags one
    tile so no engine queue head waits on same-iteration work):
    stage1: xsq via DVE STT / scalar Square-ACT (alternating parity)
      accumulated into a persistent [xsq,1] column pair, 6 bf16 PE
      transposes -> PSUM, the [128,2] pair PE-transposed to [2,128],
      PSUM->SBUF casts bf16->fp8e4 on scalar.
    stage2: 3 fp8 DoubleRow matmuls (256-deep contraction pairs) then
      one [2,512] bf16 bias matmul (xsq+csq broadcast, kept off the
      setup critical path), custom DVE 1-NR reciprocal with fused
      row-sum accumulator straight from PSUM, tiny DVE reciprocal,
      DVE normalize, sync HWDGE store.
"""

import contextlib
import os
import sys

sys.path.insert(0, "/opt/trn_rl_repo")

import numpy as np

N_CORES = 8
N, D, K = 65536, 768, 512
NC_ROWS = N // N_CORES          # 8192 rows per core
P = 128                         # partitions
N_TILES = NC_ROWS // P          # 64 row tiles per core
D_CHUNKS = D // P               # 6 contraction chunks
LOOKAHEAD = 8                   # input-DMA prefetch depth

_CACHE = {}


def _get_recip1_acc():
    """Register (once) a custom DVE op: 1-Newton-Raphson approximate
    reciprocal with a fused row-sum accumulator.  The stock
    RECIPROCAL_APPROX_FAST uses all 8 DVE stages, leaving no room for the
    accum stage; dropping one NR pass (max rel err 1.7e-3, fine for this
    kernel's 2e-2 budget) frees the stage and folds the row-sum in."""
    import concourse.dve_ops as D

    if hasattr(D, "RECIP1_ACC_ANT"):
        return D.RECIP1_ACC_ANT

    from operator import add

    from concourse.dve_ops import has_src1
    from concourse.dve_spec import AluOp, Bin, C0, C1, Spec, Src0, Zero, lower
    from concourse.dve_uop import DveOpSpec

    _not_x = Bin(AluOp.BITWISE_NOT, Src0, Src0)
    _y0 = _not_x * C0
    body = _y0 * (C1 - Src0 * _y0)

    def _ref(in0, in1, c0, c1, c2):
        not_x = (~in0.view(np.int32)).view(np.float32)
        y0 = not_x * c0
        b = (y0 * (c1 - in0 * y0)).astype(np.float32)
        return b, b.sum(axis=-1, keepdims=True, dtype=np.float32)

    spec = Spec(body=body, accum=add, accum_init=Zero, reference=_ref)
    name = "RECIP1_ACC_ANT"
    opcode = D._CUSTOM_DVE_ROW_BASE + len(D.OPS)
    assert opcode < 0x20
    shas = {}
    for ver in ("v3", "v4"):
        uops = lower(spec, ver=ver)
        shas[ver] = DveOpSpec(
            name=name, opcode=opcode, uops=uops, rd1_en=has_src1(spec)
        ).sha(ver)
    op = D.DveOp(name, spec, subdim=False, uops_sha=shas)
    D.OPS.append(op)
    D.CUSTOM_DVE_SPECS[name] = spec
    D._SUB_OPCODE_FOR_NAME[name] = opcode
    D.RECIP1_ACC_ANT = op
    return op


def _emit(nc, tc, emb_d, cen_d, out_d, alpha: float, n_rows: int):
    """Emit the per-core program into an open TileContext."""
    import concourse.bass as bass
    import concourse.mybir as mybir
    from concourse.dve_ops import RECIP_APPROX_FAST_CONSTS
    from concourse.masks import make_identity

    recip1_acc = _get_recip1_acc()

    f32 = mybir.dt.float32
    f32r = mybir.dt.float32r
    bf16 = mybir.dt.bfloat16
    f8 = mybir.dt.float8e4
    ts = bass.ts

    mm_dt = bf16 if os.environ.get("KOPT_DTYPE", "f8") == "bf16" else f8
    use_f8 = mm_dt == f8
    in_dt = bf16  # emb streams in as bf16 (fp8 PE-transpose has step-2 quirks)

    inv_a = 1.0 / alpha
    _SQRT_INV_A = float(inv_a ** 0.5)
    power = (alpha + 1.0) / 2.0
    n_tiles = n_rows // P
    reps = int(os.environ.get("KBENCH_REPS", "1"))

    with contextlib.ExitStack() as stack:
        # persistent pools first so their SBUF is disjoint from setup scratch
        const_pool = stack.enter_context(tc.tile_pool(name="const", bufs=1))
        cen_pool = stack.enter_context(tc.tile_pool(name="cent", bufs=1))
        in_pool = stack.enter_context(tc.tile_pool(name="io_in", bufs=LOOKAHEAD + 2))
        work_pool = stack.enter_context(tc.tile_pool(name="work", bufs=3))
        out_pool = stack.enter_context(tc.tile_pool(name="io_out", bufs=4))
        tp_ps = stack.enter_context(
            tc.tile_pool(name="tp_ps", bufs=2, space=bass.MemorySpace.PSUM)
        )
        xq_ps = stack.enter_context(
            tc.tile_pool(name="xq_ps", bufs=2, space=bass.MemorySpace.PSUM)
        )
        mm_ps = stack.enter_context(
            tc.tile_pool(name="mm_ps", bufs=2, space=bass.MemorySpace.PSUM)
        )

        identity = const_pool.tile([P, P], f32)
        make_identity(nc, identity[:])
        id_tp = const_pool.tile([P, P], in_dt)
        nc.scalar.copy(id_tp[:], identity[:])

        cs2 = const_pool.tile([2, K], bf16)
        nc.vector.memset(cs2[0:1, :], 1.0)
        csq1_b = const_pool.tile([1, K], bf16)

        # centersT [128, chunk, k] scaled by -2/alpha, resident, mm dtype.
        cenT = cen_pool.tile([P, D_CHUNKS, K], mm_dt, name="cenT")
        csq_cols = const_pool.tile([P, K // P], f32)
        csq_row_f32 = const_pool.tile([1, K], f32)

        with (
            tc.tile_pool(name="setup", bufs=4) as setup_pool,
            tc.tile_pool(name="setup_ps", bufs=2, space=bass.MemorySpace.PSUM) as setup_ps,
        ):
            hc = D_CHUNKS // 2
            for t in range(K // P):
                cnat = setup_pool.tile([P, D], f32, tag="cnat")
                nc.sync.dma_start(cnat[:], cen_d[ts(t, P), :])
                # csq for this block of 128 clusters (scaled by 1/alpha)
                scr = setup_pool.tile([P, D], f32, tag="cscr")
                nc.vector.scalar_tensor_tensor(
                    out=scr[:],
                    in0=cnat[:],
                    scalar=inv_a,
                    in1=cnat[:],
                    op0=mybir.AluOpType.mult,
                    op1=mybir.AluOpType.mult,
                    accum_out=csq_cols[:, t : t + 1],
                )
                # gather this block's csq column into the row right away
                nc.sync.dma_start(
                    csq_row_f32[:, ts(t, P)], csq_cols[:, t : t + 1]
                )
                # transpose+cast the 6 chunks in two double-buffered halves
                for h in range(2):
                    tps = setup_ps.tile([P, hc * P], f32, tag="tps")
                    for j in range(hc):
                        nc.tensor.transpose(
                            tps[:, ts(j, P)],
                            cnat[:, ts(h * hc + j, P)],
                            identity[:],
                        )
                    nc.scalar.mul(
                        cenT[:, h * hc : (h + 1) * hc, ts(t, P)],
                        tps.rearrange("p (c j) -> p c j", c=hc),
                        -2.0 * inv_a,
                    )
            # cs2 row 1 = 1 + csq/alpha (bf16; per-k bf16 error ~0.3% ok).
            # Engines cannot write at partition offset 1 - stage on
            # partition 0 and hop via a tiny SBUF->SBUF DMA.
            nc.scalar.activation(
                csq1_b[:],
                csq_row_f32[:],
                mybir.ActivationFunctionType.Identity,
                bias=1.0,
                scale=1.0,
            )
            nc.sync.dma_start(cs2[1:2, :], csq1_b[:])

        c = RECIP_APPROX_FAST_CONSTS

        sched = [t for _ in range(reps) for t in range(n_tiles)]
        n_iters = len(sched)
        pending = {}
        staged = {}

        def _issue_in(it_idx):
            t = in_pool.tile(
                [P, D_CHUNKS, P], in_dt, tag="emb", name=f"emb{it_idx}"
            )
            nc.gpsimd.dma_start(
                t.rearrange("p c j -> p (c j)"), emb_d[ts(sched[it_idx], P), :]
            )
            pending[it_idx] = t

        # persistent [xsq, 1] column pairs (2 slots); the accumulator writes
        # column 0 in place, column 1 stays 1.0; a PE transpose yields both
        # stationary rows of the bias matmul at once.
        xsq2_slots = []
        for s in range(2):
            t = const_pool.tile([P, 2], f32, name=f"xsq2_{s}")
            nc.vector.memset(t[:, 1:2], 1.0)
            xsq2_slots.append(t)

        def stage1(it):
            """Input prefetch + xsq + transposes + PSUM->SBUF casts."""
            if it == 0:
                for pre in range(min(LOOKAHEAD + 1, n_iters)):
                    _issue_in(pre)
            elif it + LOOKAHEAD < n_iters:
                _issue_in(it + LOOKAHEAD)
            emb8 = pending.pop(it)
            flat = emb8.rearrange("p c j -> p (c j)")

            xsq2 = xsq2_slots[it % 2]
            sq_scr = work_pool.tile([P, D], bf16, tag="sqscr")
            if it % 2 == 0:
                nc.vector.scalar_tensor_tensor(
                    out=sq_scr[:],
                    in0=flat,
                    scalar=inv_a,
                    in1=flat,
                    op0=mybir.AluOpType.mult,
                    op1=mybir.AluOpType.mult,
                    accum_out=xsq2[:, 0:1],
                )
            else:
                nc.scalar.activation(
                    sq_scr[:],
                    flat,
                    mybir.ActivationFunctionType.Square,
                    scale=_SQRT_INV_A,
                    accum_out=xsq2[:, 0:1],
                )

            tps = tp_ps.tile([P, D_CHUNKS, P], in_dt, tag="tps")
            for j in range(D_CHUNKS):
                nc.tensor.transpose(tps[:, j, :], emb8[:, j, :], id_tp[:])
            xq_row_ps = xq_ps.tile([2, P], f32, tag="xqps")
            nc.tensor.transpose(xq_row_ps[:], xsq2[:], identity[:])

            embT = work_pool.tile([P, D_CHUNKS, P], mm_dt, tag="embT")
            half = D_CHUNKS // 2
            nc.scalar.copy(embT[:, :half, :], tps[:, :half, :])
            nc.scalar.copy(embT[:, half:, :], tps[:, half:, :])
            # xq2r: [2,128] bf16 stationary - row0 = xsq row, row1 = ones
            xq2r = work_pool.tile([2, P], bf16, tag="xq2")
            nc.scalar.copy(xq2r[:], xq_row_ps[:])
            staged[it] = (embT, xq2r)

        def stage2(it):
            """Matmuls + fused reciprocal/rowsum + normalize + store."""
            embT, xq2r = staged.pop(it)
            ps = mm_ps.tile([P, K], f32, tag="u")
            # cross terms first (need only cenT), bias last (needs the csq
            # gather chain) - keeps tile 0 off the setup critical path.
            if use_f8:
                for j in range(D_CHUNKS // 2):
                    nc.tensor.matmul(
                        ps[:],
                        embT[:, 2 * j : 2 * j + 2, :],
                        cenT[:, 2 * j : 2 * j + 2, :],
                        start=(j == 0),
                        stop=False,
                        perf_mode=mybir.MatmulPerfMode.DoubleRow,
                    )
            else:
                for j in range(D_CHUNKS):
                    nc.tensor.matmul(
                        ps[:],
                        embT[:, j, :],
                        cenT[:, j, :],
                        start=(j == 0),
                        stop=False,
                    )
            nc.tensor.matmul(ps[:], xq2r[:], cs2[:], start=False, stop=True)

            numer = work_pool.tile([P, K], f32, tag="numer")
            rowsum = work_pool.tile([P, 1], f32, tag="rowsum")
            if power == 1.0:
                nc.vector._custom_dve(
                    recip1_acc,
                    out=numer[:],
                    in0=ps[:],
                    s0=c["s0"],
                    s1=c["s1"],
                    accum_out=rowsum[:],
                )
            else:
                lnd = work_pool.tile([P, K], f32, tag="lnd")
                nc.scalar.activation(
                    lnd[:], ps[:], mybir.ActivationFunctionType.Ln
                )
                nc.scalar.activation(
                    numer[:],
                    lnd[:],
                    mybir.ActivationFunctionType.Exp,
                    scale=-power,
                    accum_out=rowsum[:],
                )
            inv_rs = work_pool.tile([P, 1], f32, tag="invrs")
            nc.vector.reciprocal(inv_rs[:], rowsum[:])

            out_t = out_pool.tile([P, K], f32, tag="out")
            nc.vector.tensor_scalar_mul(out_t[:], numer[:], inv_rs[:])
            nc.sync.dma_start(out_d[ts(sched[it], P), :], out_t[:])

        # Software-pipelined: tile i's matmul/normalize stage runs one
        # iteration behind its transpose/cast stage, so no engine's queue
        # head ever waits on work issued in the same iteration.
        for it in range(n_iters):
            stage1(it)
            if it >= 1:
                stage2(it - 1)
        stage2(n_iters - 1)


def _build_program(alpha: float):
    """Standalone Bacc program (for CoreSim checks / spmd bench)."""
    import concourse.bacc as bacc
    import concourse.mybir as mybir
    import concourse.tile as tile

    f32 = mybir.dt.float32
    nc = bacc.Bacc(None, target_bir_lowering=False, debug=False, num_devices=N_CORES)
    emb_d = nc.declare_dram_parameter("embeddings", [NC_ROWS, D], f32, isOutput=False)
    cen_d = nc.declare_dram_parameter("cluster_centers", [K, D], f32, isOutput=False)
    out_d = nc.declare_dram_parameter("cluster_p", [NC_ROWS, K], f32, isOutput=True)
    with tile.TileContext(nc) as tc:
        _emit(nc, tc, emb_d, cen_d, out_d, alpha, NC_ROWS)
    nc.finalize()
    return nc


def _get_jitted(alpha: float):
    key = (float(alpha), os.environ.get("KBENCH_REPS", "1"), os.environ.get("KOPT_DTYPE", "f8"))
    if key in _CACHE:
        return _CACHE[key]

    import jax
    from jax.experimental.shard_map import shard_map
    from jax.sharding import Mesh, PartitionSpec as PS

    import concourse.mybir as mybir
    import concourse.tile as tile
    from concourse.bass2jax import bass_jit

    f32 = mybir.dt.float32

    def body(nc, emb, cen):
        out_d = nc.dram_tensor(
            "cluster_p", [NC_ROWS, K], f32, kind="ExternalOutput"
        )
        with tile.TileContext(nc) as tc:
            _emit(nc, tc, emb, cen, out_d, float(alpha), NC_ROWS)
        return out_d

    f = bass_jit(body, num_devices=N_CORES)
    mesh = Mesh(np.asarray(jax.devices()[:N_CORES]), ("core",))
    sharded = shard_map(
        f,
        mesh=mesh,
        in_specs=(PS("core"), PS(None)),
        out_specs=PS("core"),
        check_rep=False,
    )
    jitted = jax.jit(sharded)
    _CACHE[key] = (jitted, mesh)
    return _CACHE[key]


def kernel(embeddings, cluster_centers, alpha):
    emb = np.ascontiguousarray(np.asarray(embeddings, dtype=np.float32))
    cen = np.ascontiguousarray(np.asarray(cluster_centers, dtype=np.float32))
    jitted, _ = _get_jitted(float(alpha))
    try:
        out = jitted(emb, cen)
        return np.asarray(out)
    except Exception:
        # transient device hiccups have been observed; retry once
        import time as _time

        _time.sleep(60)
        out = jitted(emb, cen)
        return np.asarray(out)


# revision 22
# speedup vs baseline: 1.0043x; 1.0043x over previous
"""DEC soft-assignment (student-t, row-normalized) Trainium2 Bass kernel.

q[n,k] = (1 + ||x_n - c_k||^2/alpha)^(-(alpha+1)/2), row-normalized.

Strategy (8 cores, data-parallel over N), DMA-roofline oriented
(~160us vs ~121us pure-DMA floor for the 42MB/core of HBM traffic):
  u = 1 + (||x||^2 - 2 x.c + ||c||^2)/alpha assembled fully in PSUM:
  - emb tiles stream in via gpsimd SWDGE DMA casting fp32->bf16
    in-flight (free cast), 8-tile prefetch so the input stream is hot
    from t=0 and rides through the centers setup.
  - centersT built once at setup: [128, 6, 512] fp8e4 chunk-major,
    scaled by -2/alpha; csq gathered per-block into a [1,512] row via
    SBUF->SBUF DMAs (no DRAM roundtrip); cs2 = [ones; 1+csq/a] bf16
    (partition-1 row written via a tiny SBUF->SBUF DMA hop).
  - per 128-row tile, two software-pipelined stages (stage2 lags one
    tile so no engine queue head waits on same-iteration work):
    stage1: xsq via DVE STT / scalar Square-ACT (alternating parity)
      accumulated into a persistent [xsq,1] column pair, 6 bf16 PE
      transposes -> PSUM, the [128,2] pair PE-transposed to [2,128],
      PSUM->SBUF casts bf16->fp8e4 on scalar.
    stage2: 3 fp8 DoubleRow matmuls (256-deep contraction pairs) then
      one [2,512] bf16 bias matmul (xsq+csq broadcast, kept off the
      setup critical path), custom DVE 1-NR reciprocal with fused
      row-sum accumulator straight from PSUM, tiny DVE reciprocal,
      DVE normalize, sync HWDGE store.
"""

import contextlib
import os
import sys

sys.path.insert(0, "/opt/trn_rl_repo")

import numpy as np

N_CORES = 8
N, D, K = 65536, 768, 512
NC_ROWS = N // N_CORES          # 8192 rows per core
P = 128                         # partitions
N_TILES = NC_ROWS // P          # 64 row tiles per core
D_CHUNKS = D // P               # 6 contraction chunks
LOOKAHEAD = 8                   # input-DMA prefetch depth

_CACHE = {}


def _get_recip1_acc():
    """Register (once) a custom DVE op: 1-Newton-Raphson approximate
    reciprocal with a fused row-sum accumulator.  The stock
    RECIPROCAL_APPROX_FAST uses all 8 DVE stages, leaving no room for the
    accum stage; dropping one NR pass (max rel err 1.7e-3, fine for this
    kernel's 2e-2 budget) frees the stage and folds the row-sum in."""
    import concourse.dve_ops as D

    if hasattr(D, "RECIP1_ACC_ANT"):
        return D.RECIP1_ACC_ANT

    from operator import add

    from concourse.dve_ops import has_src1
    from concourse.dve_spec import AluOp, Bin, C0, C1, Spec, Src0, Zero, lower
    from concourse.dve_uop import DveOpSpec

    _not_x = Bin(AluOp.BITWISE_NOT, Src0, Src0)
    _y0 = _not_x * C0
    body = _y0 * (C1 - Src0 * _y0)

    def _ref(in0, in1, c0, c1, c2):
        not_x = (~in0.view(np.int32)).view(np.float32)
        y0 = not_x * c0
        b = (y0 * (c1 - in0 * y0)).astype(np.float32)
        return b, b.sum(axis=-1, keepdims=True, dtype=np.float32)

    spec = Spec(body=body, accum=add, accum_init=Zero, reference=_ref)
    name = "RECIP1_ACC_ANT"
    opcode = D._CUSTOM_DVE_ROW_BASE + len(D.OPS)
    assert opcode < 0x20
    shas = {}
    for ver in ("v3", "v4"):
        uops = lower(spec, ver=ver)
        shas[ver] = DveOpSpec(
            name=name, opcode=opcode, uops=uops, rd1_en=has_src1(spec)
        ).sha(ver)
    op = D.DveOp(name, spec, subdim=False, uops_sha=shas)
    D.OPS.append(op)
    D.CUSTOM_DVE_SPECS[name] = spec
    D._SUB_OPCODE_FOR_NAME[name] = opcode
    D.RECIP1_ACC_ANT = op
    return op


def _emit(nc, tc, emb_d, cen_d, out_d, alpha: float, n_rows: int):
    """Emit the per-core program into an open TileContext."""
    import concourse.bass as bass
    import concourse.mybir as mybir
    from concourse.dve_ops import RECIP_APPROX_FAST_CONSTS
    from concourse.masks import make_identity

    recip1_acc = _get_recip1_acc()

    f32 = mybir.dt.float32
    f32r = mybir.dt.float32r
    bf16 = mybir.dt.bfloat16
    f8 = mybir.dt.float8e4
    ts = bass.ts

    mm_dt = bf16 if os.environ.get("KOPT_DTYPE", "f8") == "bf16" else f8
    use_f8 = mm_dt == f8
    in_dt = bf16  # emb streams in as bf16 (fp8 PE-transpose has step-2 quirks)

    inv_a = 1.0 / alpha
    _SQRT_INV_A = float(inv_a ** 0.5)
    power = (alpha + 1.0) / 2.0
    n_tiles = n_rows // P
    reps = int(os.environ.get("KBENCH_REPS", "1"))

    with contextlib.ExitStack() as stack:
        # persistent pools first so their SBUF is disjoint from setup scratch
        const_pool = stack.enter_context(tc.tile_pool(name="const", bufs=1))
        cen_pool = stack.enter_context(tc.tile_pool(name="cent", bufs=1))
        in_pool = stack.enter_context(tc.tile_pool(name="io_in", bufs=LOOKAHEAD + 2))
        work_pool = stack.enter_context(tc.tile_pool(name="work", bufs=3))
        out_pool = stack.enter_context(tc.tile_pool(name="io_out", bufs=6))
        tp_ps = stack.enter_context(
            tc.tile_pool(name="tp_ps", bufs=2, space=bass.MemorySpace.PSUM)
        )
        xq_ps = stack.enter_context(
            tc.tile_pool(name="xq_ps", bufs=2, space=bass.MemorySpace.PSUM)
        )
        mm_ps = stack.enter_context(
            tc.tile_pool(name="mm_ps", bufs=2, space=bass.MemorySpace.PSUM)
        )

        identity = const_pool.tile([P, P], f32)
        make_identity(nc, identity[:])
        id_tp = const_pool.tile([P, P], in_dt)
        nc.scalar.copy(id_tp[:], identity[:])

        cs2 = const_pool.tile([2, K], bf16)
        nc.vector.memset(cs2[0:1, :], 1.0)
        csq1_b = const_pool.tile([1, K], bf16)

        # centersT [128, chunk, k] scaled by -2/alpha, resident, mm dtype.
        cenT = cen_pool.tile([P, D_CHUNKS, K], mm_dt, name="cenT")
        csq_cols = const_pool.tile([P, K // P], f32)
        csq_row_f32 = const_pool.tile([1, K], f32)

        with (
            tc.tile_pool(name="setup", bufs=4) as setup_pool,
            tc.tile_pool(name="setup_ps", bufs=2, space=bass.MemorySpace.PSUM) as setup_ps,
        ):
            hc = D_CHUNKS // 2
            for t in range(K // P):
                cnat = setup_pool.tile([P, D], f32, tag="cnat")
                nc.sync.dma_start(cnat[:], cen_d[ts(t, P), :])
                # csq for this block of 128 clusters (scaled by 1/alpha)
                scr = setup_pool.tile([P, D], f32, tag="cscr")
                nc.vector.scalar_tensor_tensor(
                    out=scr[:],
                    in0=cnat[:],
                    scalar=inv_a,
                    in1=cnat[:],
                    op0=mybir.AluOpType.mult,
                    op1=mybir.AluOpType.mult,
                    accum_out=csq_cols[:, t : t + 1],
                )
                # gather this block's csq column into the row right away
                nc.sync.dma_start(
                    csq_row_f32[:, ts(t, P)], csq_cols[:, t : t + 1]
                )
                # transpose+cast the 6 chunks in two double-buffered halves
                for h in range(2):
                    tps = setup_ps.tile([P, hc * P], f32, tag="tps")
                    for j in range(hc):
                        nc.tensor.transpose(
                            tps[:, ts(j, P)],
                            cnat[:, ts(h * hc + j, P)],
                            identity[:],
                        )
                    nc.scalar.mul(
                        cenT[:, h * hc : (h + 1) * hc, ts(t, P)],
                        tps.rearrange("p (c j) -> p c j", c=hc),
                        -2.0 * inv_a,
                    )
            # cs2 row 1 = 1 + csq/alpha (bf16; per-k bf16 error ~0.3% ok).
            # Engines cannot write at partition offset 1 - stage on
            # partition 0 and hop via a tiny SBUF->SBUF DMA.
            nc.scalar.activation(
                csq1_b[:],
                csq_row_f32[:],
                mybir.ActivationFunctionType.Identity,
                bias=1.0,
                scale=1.0,
            )
            nc.sync.dma_start(cs2[1:2, :], csq1_b[:])

        c = RECIP_APPROX_FAST_CONSTS

        sched = [t for _ in range(reps) for t in range(n_tiles)]
        n_iters = len(sched)
        pending = {}
        staged = {}

        def _issue_in(it_idx):
            t = in_pool.tile(
                [P, D_CHUNKS, P], in_dt, tag="emb", name=f"emb{it_idx}"
            )
            nc.gpsimd.dma_start(
                t.rearrange("p c j -> p (c j)"), emb_d[ts(sched[it_idx], P), :]
            )
            pending[it_idx] = t

        # persistent [xsq, 1] column pairs (2 slots); the accumulator writes
        # column 0 in place, column 1 stays 1.0; a PE transpose yields both
        # stationary rows of the bias matmul at once.
        xsq2_slots = []
        for s in range(2):
            t = const_pool.tile([P, 2], f32, name=f"xsq2_{s}")
            nc.vector.memset(t[:, 1:2], 1.0)
            xsq2_slots.append(t)

        def stage1(it):
            """Input prefetch + xsq + transposes + PSUM->SBUF casts."""
            if it == 0:
                for pre in range(min(LOOKAHEAD + 1, n_iters)):
                    _issue_in(pre)
            elif it + LOOKAHEAD < n_iters:
                _issue_in(it + LOOKAHEAD)
            emb8 = pending.pop(it)
            flat = emb8.rearrange("p c j -> p (c j)")

            xsq2 = xsq2_slots[it % 2]
            sq_scr = work_pool.tile([P, D], bf16, tag="sqscr")
            if it % 2 == 0:
                nc.vector.scalar_tensor_tensor(
                    out=sq_scr[:],
                    in0=flat,
                    scalar=inv_a,
                    in1=flat,
                    op0=mybir.AluOpType.mult,
                    op1=mybir.AluOpType.mult,
                    accum_out=xsq2[:, 0:1],
                )
            else:
                nc.scalar.activation(
                    sq_scr[:],
                    flat,
                    mybir.ActivationFunctionType.Square,
                    scale=_SQRT_INV_A,
                    accum_out=xsq2[:, 0:1],
                )

            tps = tp_ps.tile([P, D_CHUNKS, P], in_dt, tag="tps")
            for j in range(D_CHUNKS):
                nc.tensor.transpose(tps[:, j, :], emb8[:, j, :], id_tp[:])
            xq_row_ps = xq_ps.tile([2, P], f32, tag="xqps")
            nc.tensor.transpose(xq_row_ps[:], xsq2[:], identity[:])

            embT = work_pool.tile([P, D_CHUNKS, P], mm_dt, tag="embT")
            half = D_CHUNKS // 2
            nc.scalar.copy(embT[:, :half, :], tps[:, :half, :])
            nc.scalar.copy(embT[:, half:, :], tps[:, half:, :])
            # xq2r: [2,128] bf16 stationary - row0 = xsq row, row1 = ones
            xq2r = work_pool.tile([2, P], bf16, tag="xq2")
            nc.scalar.copy(xq2r[:], xq_row_ps[:])
            staged[it] = (embT, xq2r)

        def stage2(it):
            """Matmuls + fused reciprocal/rowsum + normalize + store."""
            embT, xq2r = staged.pop(it)
            ps = mm_ps.tile([P, K], f32, tag="u")
            # cross terms first (need only cenT), bias last (needs the csq
            # gather chain) - keeps tile 0 off the setup critical path.
            if use_f8:
                for j in range(D_CHUNKS // 2):
                    nc.tensor.matmul(
                        ps[:],
                        embT[:, 2 * j : 2 * j + 2, :],
                        cenT[:, 2 * j : 2 * j + 2, :],
                        start=(j == 0),
                        stop=False,
                        perf_mode=mybir.MatmulPerfMode.DoubleRow,
                    )
            else:
                for j in range(D_CHUNKS):
                    nc.tensor.matmul(
                        ps[:],
                        embT[:, j, :],
                        cenT[:, j, :],
                        start=(j == 0),
                        stop=False,
                    )
            nc.tensor.matmul(ps[:], xq2r[:], cs2[:], start=False, stop=True)

            numer = work_pool.tile([P, K], f32, tag="numer")
            rowsum = work_pool.tile([P, 1], f32, tag="rowsum")
            if power == 1.0:
                nc.vector._custom_dve(
                    recip1_acc,
                    out=numer[:],
                    in0=ps[:],
                    s0=c["s0"],
                    s1=c["s1"],
                    accum_out=rowsum[:],
                )
            else:
                lnd = work_pool.tile([P, K], f32, tag="lnd")
                nc.scalar.activation(
                    lnd[:], ps[:], mybir.ActivationFunctionType.Ln
                )
                nc.scalar.activation(
                    numer[:],
                    lnd[:],
                    mybir.ActivationFunctionType.Exp,
                    scale=-power,
                    accum_out=rowsum[:],
                )
            inv_rs = work_pool.tile([P, 1], f32, tag="invrs")
            nc.vector.reciprocal(inv_rs[:], rowsum[:])

            out_t = out_pool.tile([P, K], f32, tag="out")
            nc.vector.tensor_scalar_mul(out_t[:], numer[:], inv_rs[:])
            nc.sync.dma_start(out_d[ts(sched[it], P), :], out_t[:])

        # Software-pipelined: tile i's matmul/normalize stage runs one
        # iteration behind its transpose/cast stage, so no engine's queue
        # head ever waits on work issued in the same iteration.
        for it in range(n_iters):
            stage1(it)
            if it >= 1:
                stage2(it - 1)
        stage2(n_iters - 1)


def _build_program(alpha: float):
    """Standalone Bacc program (for CoreSim checks / spmd bench)."""
    import concourse.bacc as bacc
    import concourse.mybir as mybir
    import concourse.tile as tile

    f32 = mybir.dt.float32
    nc = bacc.Bacc(None, target_bir_lowering=False, debug=False, num_devices=N_CORES)
    emb_d = nc.declare_dram_parameter("embeddings", [NC_ROWS, D], f32, isOutput=False)
    cen_d = nc.declare_dram_parameter("cluster_centers", [K, D], f32, isOutput=False)
    out_d = nc.declare_dram_parameter("cluster_p", [NC_ROWS, K], f32, isOutput=True)
    with tile.TileContext(nc) as tc:
        _emit(nc, tc, emb_d, cen_d, out_d, alpha, NC_ROWS)
    nc.finalize()
    return nc


def _get_jitted(alpha: float):
    key = (float(alpha), os.environ.get("KBENCH_REPS", "1"), os.environ.get("KOPT_DTYPE", "f8"))
    if key in _CACHE:
        return _CACHE[key]

    import jax
    from jax.experimental.shard_map import shard_map
    from jax.sharding import Mesh, PartitionSpec as PS

    import concourse.mybir as mybir
    import concourse.tile as tile
    from concourse.bass2jax import bass_jit

    f32 = mybir.dt.float32

    def body(nc, emb, cen):
        out_d = nc.dram_tensor(
            "cluster_p", [NC_ROWS, K], f32, kind="ExternalOutput"
        )
        with tile.TileContext(nc) as tc:
            _emit(nc, tc, emb, cen, out_d, float(alpha), NC_ROWS)
        return out_d

    f = bass_jit(body, num_devices=N_CORES)
    mesh = Mesh(np.asarray(jax.devices()[:N_CORES]), ("core",))
    sharded = shard_map(
        f,
        mesh=mesh,
        in_specs=(PS("core"), PS(None)),
        out_specs=PS("core"),
        check_rep=False,
    )
    jitted = jax.jit(sharded)
    _CACHE[key] = (jitted, mesh)
    return _CACHE[key]


def kernel(embeddings, cluster_centers, alpha):
    emb = np.ascontiguousarray(np.asarray(embeddings, dtype=np.float32))
    cen = np.ascontiguousarray(np.asarray(cluster_centers, dtype=np.float32))
    jitted, _ = _get_jitted(float(alpha))
    try:
        out = jitted(emb, cen)
        return np.asarray(out)
    except Exception:
        # transient device hiccups have been observed; retry once
        import time as _time

        _time.sleep(60)
        out = jitted(emb, cen)
        return np.asarray(out)


# revision 23
# speedup vs baseline: 1.0135x; 1.0092x over previous
"""DEC soft-assignment (student-t, row-normalized) Trainium2 Bass kernel.

q[n,k] = (1 + ||x_n - c_k||^2/alpha)^(-(alpha+1)/2), row-normalized.

Strategy (8 cores, data-parallel over N), DMA-roofline oriented
(~160us vs ~121us pure-DMA floor for the 42MB/core of HBM traffic):
  u = 1 + (||x||^2 - 2 x.c + ||c||^2)/alpha assembled fully in PSUM:
  - emb tiles stream in via gpsimd SWDGE DMA casting fp32->bf16
    in-flight (free cast), 8-tile prefetch so the input stream is hot
    from t=0 and rides through the centers setup.
  - centersT built once at setup: [128, 6, 512] fp8e4 chunk-major,
    scaled by -2/alpha; csq gathered per-block into a [1,512] row via
    SBUF->SBUF DMAs (no DRAM roundtrip); cs2 = [ones; 1+csq/a] bf16
    (partition-1 row written via a tiny SBUF->SBUF DMA hop).
  - per 128-row tile, two software-pipelined stages (stage2 lags one
    tile so no engine queue head waits on same-iteration work):
    stage1: xsq via DVE STT / scalar Square-ACT (alternating parity)
      accumulated into a persistent [xsq,1] column pair, 6 bf16 PE
      transposes -> PSUM, the [128,2] pair PE-transposed to [2,128],
      PSUM->SBUF casts bf16->fp8e4 on scalar.
    stage2: 3 fp8 DoubleRow matmuls (256-deep contraction pairs) then
      one [2,512] bf16 bias matmul (xsq+csq broadcast, kept off the
      setup critical path), custom DVE 1-NR reciprocal with fused
      row-sum accumulator straight from PSUM, tiny DVE reciprocal,
      DVE normalize, sync HWDGE store.
"""

import contextlib
import os
import sys

sys.path.insert(0, "/opt/trn_rl_repo")

import numpy as np

N_CORES = 8
N, D, K = 65536, 768, 512
NC_ROWS = N // N_CORES          # 8192 rows per core
P = 128                         # partitions
N_TILES = NC_ROWS // P          # 64 row tiles per core
D_CHUNKS = D // P               # 6 contraction chunks
LOOKAHEAD = 8                   # input-DMA prefetch depth

_CACHE = {}


def _get_recip1_acc():
    """Register (once) a custom DVE op: 1-Newton-Raphson approximate
    reciprocal with a fused row-sum accumulator.  The stock
    RECIPROCAL_APPROX_FAST uses all 8 DVE stages, leaving no room for the
    accum stage; dropping one NR pass (max rel err 1.7e-3, fine for this
    kernel's 2e-2 budget) frees the stage and folds the row-sum in."""
    import concourse.dve_ops as D

    if hasattr(D, "RECIP1_ACC_ANT"):
        return D.RECIP1_ACC_ANT

    from operator import add

    from concourse.dve_ops import has_src1
    from concourse.dve_spec import AluOp, Bin, C0, C1, Spec, Src0, Zero, lower
    from concourse.dve_uop import DveOpSpec

    _not_x = Bin(AluOp.BITWISE_NOT, Src0, Src0)
    _y0 = _not_x * C0
    body = _y0 * (C1 - Src0 * _y0)

    def _ref(in0, in1, c0, c1, c2):
        not_x = (~in0.view(np.int32)).view(np.float32)
        y0 = not_x * c0
        b = (y0 * (c1 - in0 * y0)).astype(np.float32)
        return b, b.sum(axis=-1, keepdims=True, dtype=np.float32)

    spec = Spec(body=body, accum=add, accum_init=Zero, reference=_ref)
    name = "RECIP1_ACC_ANT"
    opcode = D._CUSTOM_DVE_ROW_BASE + len(D.OPS)
    assert opcode < 0x20
    shas = {}
    for ver in ("v3", "v4"):
        uops = lower(spec, ver=ver)
        shas[ver] = DveOpSpec(
            name=name, opcode=opcode, uops=uops, rd1_en=has_src1(spec)
        ).sha(ver)
    op = D.DveOp(name, spec, subdim=False, uops_sha=shas)
    D.OPS.append(op)
    D.CUSTOM_DVE_SPECS[name] = spec
    D._SUB_OPCODE_FOR_NAME[name] = opcode
    D.RECIP1_ACC_ANT = op
    return op


def _emit(nc, tc, emb_d, cen_d, out_d, alpha: float, n_rows: int):
    """Emit the per-core program into an open TileContext."""
    import concourse.bass as bass
    import concourse.mybir as mybir
    from concourse.dve_ops import RECIP_APPROX_FAST_CONSTS
    from concourse.masks import make_identity

    recip1_acc = _get_recip1_acc()

    f32 = mybir.dt.float32
    f32r = mybir.dt.float32r
    bf16 = mybir.dt.bfloat16
    f8 = mybir.dt.float8e4
    ts = bass.ts

    mm_dt = bf16 if os.environ.get("KOPT_DTYPE", "f8") == "bf16" else f8
    use_f8 = mm_dt == f8
    in_dt = bf16  # emb streams in as bf16 (fp8 PE-transpose has step-2 quirks)

    inv_a = 1.0 / alpha
    _SQRT_INV_A = float(inv_a ** 0.5)
    power = (alpha + 1.0) / 2.0
    n_tiles = n_rows // P
    reps = int(os.environ.get("KBENCH_REPS", "1"))

    with contextlib.ExitStack() as stack:
        # persistent pools first so their SBUF is disjoint from setup scratch
        const_pool = stack.enter_context(tc.tile_pool(name="const", bufs=1))
        cen_pool = stack.enter_context(tc.tile_pool(name="cent", bufs=1))
        in_pool = stack.enter_context(tc.tile_pool(name="io_in", bufs=LOOKAHEAD + 2))
        work_pool = stack.enter_context(tc.tile_pool(name="work", bufs=3))
        out_pool = stack.enter_context(tc.tile_pool(name="io_out", bufs=6))
        tp_ps = stack.enter_context(
            tc.tile_pool(name="tp_ps", bufs=2, space=bass.MemorySpace.PSUM)
        )
        xq_ps = stack.enter_context(
            tc.tile_pool(name="xq_ps", bufs=2, space=bass.MemorySpace.PSUM)
        )
        mm_ps = stack.enter_context(
            tc.tile_pool(name="mm_ps", bufs=2, space=bass.MemorySpace.PSUM)
        )

        identity = const_pool.tile([P, P], f32)
        make_identity(nc, identity[:])
        id_tp = const_pool.tile([P, P], in_dt)
        nc.scalar.copy(id_tp[:], identity[:])

        cs2 = const_pool.tile([2, K], bf16)
        nc.vector.memset(cs2[0:1, :], 1.0)
        csq1_b = const_pool.tile([1, K], bf16)

        # centersT [128, chunk, k] scaled by -2/alpha, resident, mm dtype.
        cenT = cen_pool.tile([P, D_CHUNKS, K], mm_dt, name="cenT")
        csq_cols = const_pool.tile([P, K // P], f32)
        csq_row_f32 = const_pool.tile([1, K], f32)

        with (
            tc.tile_pool(name="setup", bufs=4) as setup_pool,
            tc.tile_pool(name="setup_ps", bufs=2, space=bass.MemorySpace.PSUM) as setup_ps,
        ):
            hc = D_CHUNKS // 2
            cnats = []
            for t in range(K // P):
                cnat = setup_pool.tile([P, D], f32, tag="cnat")
                nc.sync.dma_start(cnat[:], cen_d[ts(t, P), :])
                cnats.append(cnat)
                # csq for this block of 128 clusters (scaled by 1/alpha)
                scr = setup_pool.tile([P, D], f32, tag="cscr")
                nc.vector.scalar_tensor_tensor(
                    out=scr[:],
                    in0=cnat[:],
                    scalar=inv_a,
                    in1=cnat[:],
                    op0=mybir.AluOpType.mult,
                    op1=mybir.AluOpType.mult,
                    accum_out=csq_cols[:, t : t + 1],
                )
                # gather this block's csq column into the row right away
                nc.sync.dma_start(
                    csq_row_f32[:, ts(t, P)], csq_cols[:, t : t + 1]
                )
            # transpose+cast chunk-half-major across all 4 k-blocks so the
            # first DoubleRow pair (chunks 0,1) is complete after half the
            # setup compute - subtile deps let tile 0's first matmuls start
            # before the whole cenT is built.
            for h in range(2):
                for t in range(K // P):
                    tps = setup_ps.tile([P, hc * P], f32, tag="tps")
                    for j in range(hc):
                        nc.tensor.transpose(
                            tps[:, ts(j, P)],
                            cnats[t][:, ts(h * hc + j, P)],
                            identity[:],
                        )
                    nc.scalar.mul(
                        cenT[:, h * hc : (h + 1) * hc, ts(t, P)],
                        tps.rearrange("p (c j) -> p c j", c=hc),
                        -2.0 * inv_a,
                    )
            # cs2 row 1 = 1 + csq/alpha (bf16; per-k bf16 error ~0.3% ok).
            # Engines cannot write at partition offset 1 - stage on
            # partition 0 and hop via a tiny SBUF->SBUF DMA.
            nc.scalar.activation(
                csq1_b[:],
                csq_row_f32[:],
                mybir.ActivationFunctionType.Identity,
                bias=1.0,
                scale=1.0,
            )
            nc.sync.dma_start(cs2[1:2, :], csq1_b[:])

        c = RECIP_APPROX_FAST_CONSTS

        sched = [t for _ in range(reps) for t in range(n_tiles)]
        n_iters = len(sched)
        pending = {}
        staged = {}

        def _issue_in(it_idx):
            t = in_pool.tile(
                [P, D_CHUNKS, P], in_dt, tag="emb", name=f"emb{it_idx}"
            )
            nc.gpsimd.dma_start(
                t.rearrange("p c j -> p (c j)"), emb_d[ts(sched[it_idx], P), :]
            )
            pending[it_idx] = t

        # persistent [xsq, 1] column pairs (2 slots); the accumulator writes
        # column 0 in place, column 1 stays 1.0; a PE transpose yields both
        # stationary rows of the bias matmul at once.
        xsq2_slots = []
        for s in range(2):
            t = const_pool.tile([P, 2], f32, name=f"xsq2_{s}")
            nc.vector.memset(t[:, 1:2], 1.0)
            xsq2_slots.append(t)

        def stage1(it):
            """Input prefetch + xsq + transposes + PSUM->SBUF casts."""
            if it == 0:
                for pre in range(min(LOOKAHEAD + 1, n_iters)):
                    _issue_in(pre)
            elif it + LOOKAHEAD < n_iters:
                _issue_in(it + LOOKAHEAD)
            emb8 = pending.pop(it)
            flat = emb8.rearrange("p c j -> p (c j)")

            xsq2 = xsq2_slots[it % 2]
            sq_scr = work_pool.tile([P, D], bf16, tag="sqscr")
            if it % 2 == 0:
                nc.vector.scalar_tensor_tensor(
                    out=sq_scr[:],
                    in0=flat,
                    scalar=inv_a,
                    in1=flat,
                    op0=mybir.AluOpType.mult,
                    op1=mybir.AluOpType.mult,
                    accum_out=xsq2[:, 0:1],
                )
            else:
                nc.scalar.activation(
                    sq_scr[:],
                    flat,
                    mybir.ActivationFunctionType.Square,
                    scale=_SQRT_INV_A,
                    accum_out=xsq2[:, 0:1],
                )

            tps = tp_ps.tile([P, D_CHUNKS, P], in_dt, tag="tps")
            for j in range(D_CHUNKS):
                nc.tensor.transpose(tps[:, j, :], emb8[:, j, :], id_tp[:])
            xq_row_ps = xq_ps.tile([2, P], f32, tag="xqps")
            nc.tensor.transpose(xq_row_ps[:], xsq2[:], identity[:])

            embT = work_pool.tile([P, D_CHUNKS, P], mm_dt, tag="embT")
            half = D_CHUNKS // 2
            nc.scalar.copy(embT[:, :half, :], tps[:, :half, :])
            nc.scalar.copy(embT[:, half:, :], tps[:, half:, :])
            # xq2r: [2,128] bf16 stationary - row0 = xsq row, row1 = ones
            xq2r = work_pool.tile([2, P], bf16, tag="xq2")
            nc.scalar.copy(xq2r[:], xq_row_ps[:])
            staged[it] = (embT, xq2r)

        def stage2(it):
            """Matmuls + fused reciprocal/rowsum + normalize + store."""
            embT, xq2r = staged.pop(it)
            ps = mm_ps.tile([P, K], f32, tag="u")
            # cross terms first (need only cenT), bias last (needs the csq
            # gather chain) - keeps tile 0 off the setup critical path.
            if use_f8:
                for j in range(D_CHUNKS // 2):
                    nc.tensor.matmul(
                        ps[:],
                        embT[:, 2 * j : 2 * j + 2, :],
                        cenT[:, 2 * j : 2 * j + 2, :],
                        start=(j == 0),
                        stop=False,
                        perf_mode=mybir.MatmulPerfMode.DoubleRow,
                    )
            else:
                for j in range(D_CHUNKS):
                    nc.tensor.matmul(
                        ps[:],
                        embT[:, j, :],
                        cenT[:, j, :],
                        start=(j == 0),
                        stop=False,
                    )
            nc.tensor.matmul(ps[:], xq2r[:], cs2[:], start=False, stop=True)

            numer = work_pool.tile([P, K], f32, tag="numer")
            rowsum = work_pool.tile([P, 1], f32, tag="rowsum")
            if power == 1.0:
                nc.vector._custom_dve(
                    recip1_acc,
                    out=numer[:],
                    in0=ps[:],
                    s0=c["s0"],
                    s1=c["s1"],
                    accum_out=rowsum[:],
                )
            else:
                lnd = work_pool.tile([P, K], f32, tag="lnd")
                nc.scalar.activation(
                    lnd[:], ps[:], mybir.ActivationFunctionType.Ln
                )
                nc.scalar.activation(
                    numer[:],
                    lnd[:],
                    mybir.ActivationFunctionType.Exp,
                    scale=-power,
                    accum_out=rowsum[:],
                )
            inv_rs = work_pool.tile([P, 1], f32, tag="invrs")
            nc.vector.reciprocal(inv_rs[:], rowsum[:])

            out_t = out_pool.tile([P, K], f32, tag="out")
            nc.vector.tensor_scalar_mul(out_t[:], numer[:], inv_rs[:])
            nc.sync.dma_start(out_d[ts(sched[it], P), :], out_t[:])

        # Software-pipelined: tile i's matmul/normalize stage runs one
        # iteration behind its transpose/cast stage, so no engine's queue
        # head ever waits on work issued in the same iteration.
        for it in range(n_iters):
            stage1(it)
            if it >= 1:
                stage2(it - 1)
        stage2(n_iters - 1)


def _build_program(alpha: float):
    """Standalone Bacc program (for CoreSim checks / spmd bench)."""
    import concourse.bacc as bacc
    import concourse.mybir as mybir
    import concourse.tile as tile

    f32 = mybir.dt.float32
    nc = bacc.Bacc(None, target_bir_lowering=False, debug=False, num_devices=N_CORES)
    emb_d = nc.declare_dram_parameter("embeddings", [NC_ROWS, D], f32, isOutput=False)
    cen_d = nc.declare_dram_parameter("cluster_centers", [K, D], f32, isOutput=False)
    out_d = nc.declare_dram_parameter("cluster_p", [NC_ROWS, K], f32, isOutput=True)
    with tile.TileContext(nc) as tc:
        _emit(nc, tc, emb_d, cen_d, out_d, alpha, NC_ROWS)
    nc.finalize()
    return nc


def _get_jitted(alpha: float):
    key = (float(alpha), os.environ.get("KBENCH_REPS", "1"), os.environ.get("KOPT_DTYPE", "f8"))
    if key in _CACHE:
        return _CACHE[key]

    import jax
    from jax.experimental.shard_map import shard_map
    from jax.sharding import Mesh, PartitionSpec as PS

    import concourse.mybir as mybir
    import concourse.tile as tile
    from concourse.bass2jax import bass_jit

    f32 = mybir.dt.float32

    def body(nc, emb, cen):
        out_d = nc.dram_tensor(
            "cluster_p", [NC_ROWS, K], f32, kind="ExternalOutput"
        )
        with tile.TileContext(nc) as tc:
            _emit(nc, tc, emb, cen, out_d, float(alpha), NC_ROWS)
        return out_d

    f = bass_jit(body, num_devices=N_CORES)
    mesh = Mesh(np.asarray(jax.devices()[:N_CORES]), ("core",))
    sharded = shard_map(
        f,
        mesh=mesh,
        in_specs=(PS("core"), PS(None)),
        out_specs=PS("core"),
        check_rep=False,
    )
    jitted = jax.jit(sharded)
    _CACHE[key] = (jitted, mesh)
    return _CACHE[key]


def kernel(embeddings, cluster_centers, alpha):
    emb = np.ascontiguousarray(np.asarray(embeddings, dtype=np.float32))
    cen = np.ascontiguousarray(np.asarray(cluster_centers, dtype=np.float32))
    jitted, _ = _get_jitted(float(alpha))
    try:
        out = jitted(emb, cen)
        return np.asarray(out)
    except Exception:
        # transient device hiccups have been observed; retry once
        import time as _time

        _time.sleep(60)
        out = jitted(emb, cen)
        return np.asarray(out)
